# revision 28
# baseline (speedup 1.0000x reference)
"""Llama MHA (B=2, S=2048, D=2048, H=16, causal, RoPE) on 8 trn2 cores.

Sharding: data-parallel over batch (2 groups of 4 cores) x tensor-parallel
over heads (4 heads per core). Each core computes, for its (batch, 4 heads):
  qT/kT = w^T x^T  (features on partitions, seq on free dim)
  RoPE on qT/kT (weights column-permuted on host so even/odd feature pairs
  land de-interleaved; dot products are permutation-invariant so scores
  match the reference exactly). RoPE runs as one Scalar-engine PSUM drain
  to fp16 followed by all-fp16 SBUF DVE ops (4x DVE mode).
  scoresT[k,q] blocks -> exp(fp16). Causal handling: diagonal-superblock
  matmuls are narrowed to the valid query range (saves ~10% of attention
  PE cycles) and a single shared [128,128] triangular mask covers the
  128-wide diagonal window of each key block.
  Softmax denominator: pt blocks are accumulated per (head, qchunk) with
  cheap fp16 DVE adds, then ONE [128,128]-ones matmul per (head, qchunk)
  does the partition reduction with a free PSUM row-broadcast (replaces
  the per-block ones-matmuls: ~82K -> ~8K PE cycles).
  Scheduling: the attention block loop is exp-throughput-bound on the
  Scalar engine, so chunk qc's out-projection is interleaved at block
  granularity into chunk qc+1's attention, each block's PV matmul is
  deferred one block (depth-2 software pipeline), and chunk 0's attention
  is interleaved into the projection of chunks 1-2. Scratch matmuls keep
  the PE's HAM activity monitor warm through the DMA-bound ramp. Partial
  results ship as fp16 and the host sums the 4 partials per batch in fp32.

All matmuls in fp16 (fp32 PSUM accumulation); softmax/normalization fp32.
"""

import numpy as np

import concourse.bass as bass
import concourse.mybir as mybir
import concourse.tile as tile
from concourse import bacc
from concourse.bass_utils import run_bass_kernel_spmd

B, S, D, H = 2, 2048, 2048, 16
DH = D // H            # 128 head dim
HPC = 4                # heads per core
N_CORES = 8
FH = HPC * DH          # 512 features per core
P = 128
KT = D // P            # 16 k-tiles over D
SC = S // 512          # 4 seq chunks of 512
ST = S // P            # 16 seq blocks of 128
THETA = 10000.0
SCALE = 1.0 / np.sqrt(DH)

DT = mybir.dt.float16
NPDT = np.float16

_prog_cache = {}


def _build():
    if "nc" in _prog_cache:
        return _prog_cache["nc"]
    nc = bacc.Bacc(None, target_bir_lowering=False, debug=False)

    xT = nc.dram_tensor("xT", [D, S], DT, kind="ExternalInput")
    wq = nc.dram_tensor("wq", [D, FH], DT, kind="ExternalInput")
    wk = nc.dram_tensor("wk", [D, FH], DT, kind="ExternalInput")
    wv = nc.dram_tensor("wv", [D, FH], DT, kind="ExternalInput")
    wo = nc.dram_tensor("wo", [FH, D], DT, kind="ExternalInput")
    cc = nc.dram_tensor("cc", [P, S], DT, kind="ExternalInput")
    ss = nc.dram_tensor("ss", [P, S], DT, kind="ExternalInput")
    masks = nc.dram_tensor("masks", [P, P], DT, kind="ExternalInput")
    resT = nc.dram_tensor("resT", [D, S], DT, kind="ExternalOutput")

    f32 = mybir.dt.float32
    Exp = mybir.ActivationFunctionType.Exp
    Copy = mybir.ActivationFunctionType.Copy

    with tile.TileContext(nc) as tc:
        with (
            tc.tile_pool(name="persist", bufs=1) as pp,
            tc.tile_pool(name="psA", bufs=5, space="PSUM") as psA,
            tc.tile_pool(name="psO", bufs=2, space="PSUM") as psO,
        ):
            qT = pp.tile([P, HPC, S], DT)     # per head: rows=feat, free=seq
            kT = pp.tile([P, HPC, S], DT)
            vn = pp.tile([P, ST, FH], DT)     # v natural: [seq-block, feat]
            attnT = pp.tile([P, HPC, S], DT)  # normalized attention output^T
            cc_t = pp.tile([P, S], DT)
            ss_t = pp.tile([P, S], DT)
            mask_t = pp.tile([P, P], DT)      # tri mask: 1 if k <= q (local)
            ones_mat = pp.tile([P, P], DT)    # denominator reduce stationary
            wo_t = pp.tile([P, HPC, D], DT)

            nc.vector.memset(ones_mat, 1.0)

            # ---------------- Phase 1: projections + RoPE -----------------
            with (
                tc.tile_pool(name="wpool", bufs=1) as wp,
                tc.tile_pool(name="xpool", bufs=2) as xp,
                tc.tile_pool(name="ropetmp", bufs=6) as rp,
                tc.tile_pool(name="warm", bufs=1) as wmp,
            ):
                # HAM pre-warm: full-width matmuls on scratch data keep the
                # PE array genuinely busy through the ~3.4us activity window
                # while the input DMAs stage, so the first real matmuls run
                # at full clock. (Narrow matmuls don't register enough
                # activity — the array idles between dispatches.)
                wsb = wmp.tile([P, 256], DT)
                wps = psA.tile([P, 256], f32, tag="ps", name="warm")
                nc.vector.memset(wsb, 0.0)
                for _ in range(16):
                    nc.tensor.matmul(wps, wsb[:, 0:P], wsb, start=True, stop=True)

                wq_t = wp.tile([P, KT, FH], DT)
                wk_t = wp.tile([P, KT, FH], DT)
                wv_t = wp.tile([P, KT, FH], DT)
                # DMA issue order is the Sync-queue order: stage the first x
                # chunk and wq head-by-head so the first q-chain (head 0) can
                # start after ~2.5MB instead of 4MB; defer everything not
                # needed immediately.
                # Weights stream on the GpSimd DGE queue, activations on the
                # SP queue — two hardware DMA paths in parallel through the
                # DMA-bound ramp.
                xc0 = xp.tile([P, KT, 512], DT, tag="xc", name="xc0")
                wqr = wq.rearrange("(kt p) f -> p kt f", p=P)
                for gs in (slice(0, 2), slice(2, 4), slice(4, 8),
                           slice(8, 12), slice(12, 16)):
                    nc.gpsimd.dma_start(out=wq_t[:, gs, 0:DH], in_=wqr[:, gs, 0:DH])
                    nc.sync.dma_start(
                        out=xc0[:, gs, :],
                        in_=xT.rearrange("(kt p) s -> p kt s", p=P)[:, gs, 0:512])
                for h in range(1, HPC):
                    fsl = slice(h * DH, (h + 1) * DH)
                    nc.gpsimd.dma_start(out=wq_t[:, :, fsl], in_=wqr[:, :, fsl])
                nc.sync.dma_start(out=cc_t[:, 0:512], in_=cc[:, 0:512])
                nc.sync.dma_start(out=ss_t[:, 0:512], in_=ss[:, 0:512])
                for g in range(4):
                    gs = slice(g * 4, (g + 1) * 4)
                    nc.gpsimd.dma_start(
                        out=wk_t[:, gs, :],
                        in_=wk.rearrange("(kt p) f -> p kt f", p=P)[:, gs, :])
                nc.gpsimd.dma_start(
                    out=wv_t, in_=wv.rearrange("(kt p) f -> p kt f", p=P))
                nc.sync.dma_start(out=mask_t, in_=masks[:, :])
                nc.sync.dma_start(out=cc_t[:, 512:], in_=cc[:, 512:])
                nc.sync.dma_start(out=ss_t[:, 512:], in_=ss[:, 512:])

                # Attention for query-chunk 0 is interleaved into the
                # projection of chunks 1..2 (it only needs chunk-0 q/k/v,
                # all ready after sc=0). Each block is split into a `pre`
                # (score matmul + exp + mask) and `post` (PV + running
                # denominator) emitted around a full projection chain, so
                # the ~600ns exp hides under ~3.5us of projection matmuls.
                def build_attn0_steps():
                    steps = []
                    for h in range(HPC):
                        fsl = slice(h * DH, (h + 1) * DH)
                        st = {}
                        for kb in range(4):
                            off = 128 * kb if kb > 0 else 0
                            wsl = slice(off, 512)

                            def pre(h=h, kb=kb, off=off, wsl=wsl, st=st):
                                if kb == 0:
                                    st["po"] = psO.tile(
                                        [P, 512], f32, tag="po", name=f"a0po{h}")
                                    st["pda"] = rp.tile(
                                        [P, 512], DT, tag="pda0", bufs=2,
                                        name=f"a0pda{h}")
                                ps = psA.tile([P, 512], f32, tag="ps",
                                              name=f"a0ps{h}{kb}")
                                nc.tensor.matmul(
                                    ps[:, wsl], kT[:, h, kb * P:(kb + 1) * P],
                                    qT[:, h, off:512], start=True, stop=True)
                                pt = rp.tile([P, 512], DT, tag="pt0", bufs=4,
                                             name=f"a0pt{h}{kb}")
                                nc.scalar.activation(
                                    pt[:, wsl], ps[:, wsl], Exp,
                                    scale=float(SCALE))
                                nc.vector.tensor_mul(
                                    pt[:, off:off + P], pt[:, off:off + P],
                                    mask_t)
                                st["pt"] = pt

                            def post(h=h, kb=kb, off=off, wsl=wsl, st=st,
                                     fsl=fsl):
                                pt, po, pda = st["pt"], st["po"], st["pda"]
                                nc.tensor.matmul(
                                    po[:, wsl], vn[:, kb, fsl], pt[:, wsl],
                                    start=(kb == 0), stop=(kb == 3),
                                    skip_group_check=True)
                                if kb == 0:
                                    nc.vector.tensor_copy(pda, pt)
                                else:
                                    nc.vector.tensor_add(
                                        pda[:, wsl], pda[:, wsl], pt[:, wsl])

                            steps.append((pre, post))

                        def fin(h=h, st=st):
                            pd = psO.tile([P, 512], f32, tag="pd", bufs=1,
                                          name=f"a0pd{h}")
                            nc.tensor.matmul(pd, ones_mat, st["pda"],
                                             start=True, stop=True)
                            bc = rp.tile([P, 512], f32, tag="bc0", bufs=2,
                                         name=f"a0bc{h}")
                            nc.vector.reciprocal_approx_fast(out=bc, in_=pd)
                            nc.vector.tensor_mul(
                                attnT[:, h, 0:512], st["po"], bc)

                        steps.append((fin, None))
                    return steps

                attn0_steps = build_attn0_steps()
                attn0_idx = [0]

                def attn0_pre():
                    if attn0_idx[0] < len(attn0_steps):
                        attn0_steps[attn0_idx[0]][0]()

                def attn0_post():
                    if attn0_idx[0] < len(attn0_steps):
                        post = attn0_steps[attn0_idx[0]][1]
                        if post is not None:
                            post()
                        attn0_idx[0] += 1

                for sc in range(SC):
                    if sc == 0:
                        xc = xc0
                    else:
                        xc = xp.tile([P, KT, 512], DT, tag="xc", name=f"xc{sc}")
                        for g in range(4):
                            gs = slice(g * 4, (g + 1) * 4)
                            nc.sync.dma_start(
                                out=xc[:, gs, :],
                                in_=xT.rearrange("(kt p) s -> p kt s", p=P)[
                                    :, gs, sc * 512:(sc + 1) * 512],
                            )
                    if sc == 1:
                        nc.gpsimd.dma_start(
                            out=wo_t, in_=wo.rearrange("(ft p) d -> p ft d", p=P))
                    csl = slice(sc * 512, (sc + 1) * 512)
                    # q/k projections. RoPE fused into the PSUM drain:
                    # Scalar engine copies PSUM->fp16 SBUF, then three
                    # all-fp16 DVE ops (eligible for the fast DVE mode).
                    # ss_t rows 0:64 = +sin, rows 64:128 = -sin; the swap is
                    # done by writing each product into the opposite half.
                    for ci, (wt, dst, h) in enumerate(
                            [(w, d, hh) for w, d in ((wq_t, qT), (wk_t, kT))
                             for hh in range(HPC)]):
                        fsl = slice(h * DH, (h + 1) * DH)
                        if sc >= 1:
                            attn0_pre()
                        pq = psA.tile([P, 512], f32, tag="ps", name=f"pq{sc}{h}{ci}")
                        for k in range(KT):
                            nc.tensor.matmul(
                                pq, wt[:, k, fsl], xc[:, k, :],
                                start=(k == 0), stop=(k == KT - 1),
                            )
                            if sc == 0 and ci < 4 and k % 4 == 3:
                                # scratch matmul: keeps HAM warm through the
                                # DMA-bound stretch of the first chains
                                nc.tensor.matmul(wps, wsb[:, 0:P], wsb,
                                                 start=True, stop=True)
                        if sc >= 1:
                            attn0_post()
                        pqs = rp.tile([P, 512], DT, tag="pqs")
                        ta = rp.tile([P, 512], DT, tag="ta")
                        tb = rp.tile([P, 512], DT, tag="tb")
                        nc.scalar.activation(pqs, pq, Copy)
                        nc.vector.tensor_mul(ta, pqs, cc_t[:, csl])
                        nc.vector.tensor_mul(
                            tb[0:64, :], pqs[64:128, :], ss_t[64:128, csl])
                        nc.vector.tensor_mul(
                            tb[64:128, :], pqs[0:64, :], ss_t[0:64, csl])
                        nc.vector.tensor_add(dst[:, h, csl], ta, tb)
                    # v projection straight into natural layout
                    for st4 in range(4):
                        sb = sc * 4 + st4
                        if sc >= 1:
                            attn0_pre()
                        pv = psA.tile([P, FH], f32, tag="ps", name=f"pv{sc}{st4}")
                        for k in range(KT):
                            nc.tensor.matmul(
                                pv, xc[:, k, st4 * P:(st4 + 1) * P], wv_t[:, k, :],
                                start=(k == 0), stop=(k == KT - 1),
                            )
                        if sc >= 1:
                            attn0_post()
                        nc.vector.tensor_copy(vn[:, sb, :], pv)
                assert attn0_idx[0] >= len(attn0_steps), "attn0 steps left over"

            # ---------- Phase 2+3: attention + interleaved out-proj -------
            # The attention inner loop is exp-throughput-bound on the Scalar
            # engine (~600ns/block vs ~430ns of PE work), so the PE would
            # stall on every PV matmul waiting for its exp. The previous
            # chunk's out-projection is dependency-free by then: its db
            # blocks are injected INTO the block loop (one per few blocks)
            # to fill those stalls. The final chunk's out-projection runs
            # standalone at the end (nothing left to overlap with).
            with (
                tc.tile_pool(name="ppool", bufs=8) as ptp,
                tc.tile_pool(name="rpool", bufs=2) as rop,
            ):
                resTr = resT.rearrange("(db p) s -> p db s", p=P)

                def outproj_units(qc):
                    """One unit per db: 4 matmuls + drain (+ paired DMA)."""
                    qsl = slice(qc * 512, (qc + 1) * 512)
                    rt = rop.tile([P, KT, 512], DT, tag="rt", name=f"rt{qc}")

                    def unit(db):
                        pr = psA.tile([P, 512], f32, tag="ps", name=f"pr{qc}{db}")
                        for ft in range(HPC):
                            nc.tensor.matmul(
                                pr, wo_t[:, ft, db * P:(db + 1) * P],
                                attnT[:, ft, qsl],
                                start=(ft == 0), stop=(ft == HPC - 1),
                            )
                        if db % 4 == 3:
                            nc.scalar.activation(rt[:, db, :], pr, Copy)
                        else:
                            nc.vector.tensor_copy(rt[:, db, :], pr)
                        if db % 2 == 1:
                            nc.sync.dma_start(
                                out=resTr[:, db - 1:db + 1, qsl],
                                in_=rt[:, db - 1:db + 1, :])

                    return [lambda db=db: unit(db) for db in range(KT)]

                # Depth-2 software pipeline: block k's PV matmul is emitted
                # during block k+1 (after its score matmul and any injected
                # out-proj unit), so the PE never sits in-order behind the
                # ~600ns exp. Each head's denominator finalize (ones-matmul,
                # reciprocal, normalize) is likewise deferred into the next
                # head. `pend` carries across heads and chunks.
                pend = {"pv": None, "fin": None}

                def flush_pv():
                    if pend["pv"] is not None:
                        pend["pv"]()
                        pend["pv"] = None

                def flush_fin():
                    if pend["fin"] is not None:
                        pend["fin"]()
                        pend["fin"] = None

                def attention_chunk(qc, inject):
                    qsl = slice(qc * 512, (qc + 1) * 512)
                    nkb = 4 * qc + 4
                    nblocks = HPC * nkb
                    # spread the injected units evenly across the block loop
                    inj_at = {}
                    for i, u in enumerate(inject):
                        inj_at.setdefault(
                            min(nblocks - 1, (i * nblocks) // len(inject) + 1),
                            []).append(u)
                    blk = 0
                    for h in range(HPC):
                        fsl = slice(h * DH, (h + 1) * DH)
                        po = psO.tile([P, 512], f32, tag="po", name=f"po{h}{qc}")
                        pda = ptp.tile([P, 512], DT, tag="pda", bufs=3,
                                       name=f"pda{h}{qc}")
                        for kb in range(nkb):
                            jl = kb - 4 * qc       # >=0 only on the diagonal
                            off = 128 * jl if jl > 0 else 0
                            wsl = slice(off, 512)
                            ps = psA.tile([P, 512], f32, tag="ps",
                                          name=f"ps{h}{qc}{kb}")
                            nc.tensor.matmul(
                                ps[:, wsl], kT[:, h, kb * P:(kb + 1) * P],
                                qT[:, h, qc * 512 + off:(qc + 1) * 512],
                                start=True, stop=True,
                            )
                            pt = ptp.tile([P, 512], DT, tag="pt")
                            nc.scalar.activation(
                                pt[:, wsl], ps[:, wsl], Exp, scale=float(SCALE))
                            if jl >= 0:
                                # triangular mask on the 128-wide diag window
                                nc.vector.tensor_mul(
                                    pt[:, off:off + P], pt[:, off:off + P],
                                    mask_t)
                            flush_pv()
                            if kb == 1:
                                # previous head's/chunk's finalize runs (and
                                # writes attnT) before any injected out-proj
                                # unit can read that attnT chunk
                                flush_fin()
                            for u in inj_at.get(blk, ()):
                                u()

                            def pv_unit(po=po, pda=pda, pt=pt, kb=kb, wsl=wsl,
                                        fsl=fsl, nkb=nkb):
                                nc.tensor.matmul(
                                    po[:, wsl], vn[:, kb, fsl], pt[:, wsl],
                                    start=(kb == 0), stop=(kb == nkb - 1),
                                    skip_group_check=True,
                                )
                                if kb == 0:
                                    nc.vector.tensor_copy(pda, pt)
                                else:
                                    nc.vector.tensor_add(
                                        pda[:, wsl], pda[:, wsl], pt[:, wsl])

                            pend["pv"] = pv_unit
                            blk += 1

                        def fin_unit(h=h, qc=qc, po=po, pda=pda, qsl=qsl):
                            pd = psO.tile([P, 512], f32, tag="pd", bufs=1,
                                          name=f"pd{h}{qc}")
                            nc.tensor.matmul(pd, ones_mat, pda,
                                             start=True, stop=True)
                            bc = ptp.tile([P, 512], f32, tag="bc", bufs=4)
                            nc.vector.reciprocal_approx_fast(out=bc, in_=pd)
                            nc.vector.tensor_mul(attnT[:, h, qsl], po, bc)

                        flush_fin()
                        pend["fin"] = fin_unit

                # chunk 0's attention already ran inside phase 1
                for qc in range(1, SC):
                    attention_chunk(qc, outproj_units(qc - 1))
                flush_pv()
                flush_fin()
                for u in outproj_units(SC - 1):
                    u()

    nc.finalize()
    _prog_cache["nc"] = nc
    return nc


def _host_inputs(x, w_q, w_k, w_v, w_o):
    """Build the 8 per-core input maps."""
    # RoPE de-interleave permutation per head: evens then odds
    i = np.arange(DH)
    perm_head = np.concatenate([i[0::2], i[1::2]])  # within-head column order

    t = np.arange(S, dtype=np.float64)
    inv_freq = 1.0 / (THETA ** (np.arange(0, DH, 2, dtype=np.float64) / DH))
    ang = np.outer(t, inv_freq)          # [S, 64]
    cosT = np.cos(ang).T.astype(np.float32)   # [64, S]
    sinT = np.sin(ang).T.astype(np.float32)
    cc = np.vstack([cosT, cosT]).astype(NPDT)   # [128, S]
    ss = np.vstack([sinT, -sinT]).astype(NPDT)  # +sin feeds bottom half

    # shared diagonal mask: mask[k, q] = 1 if k <= q (128-wide local window)
    kk = np.arange(P)[:, None]
    qq = np.arange(P)[None, :]
    masks = (kk <= qq).astype(NPDT)      # [128, 128]

    in_maps = []
    for core in range(N_CORES):
        b = core // 4
        h0 = (core % 4) * HPC
        cols = np.concatenate(
            [h * DH + perm_head for h in range(h0, h0 + HPC)])   # rope-permuted
        vcols = np.arange(h0 * DH, (h0 + HPC) * DH)              # natural
        in_maps.append({
            "xT": np.ascontiguousarray(x[b].T).astype(NPDT),
            "wq": np.ascontiguousarray(w_q[:, cols]).astype(NPDT),
            "wk": np.ascontiguousarray(w_k[:, cols]).astype(NPDT),
            "wv": np.ascontiguousarray(w_v[:, vcols]).astype(NPDT),
            "wo": np.ascontiguousarray(w_o[vcols, :]).astype(NPDT),
            "cc": cc,
            "ss": ss,
            "masks": masks,
        })
    return in_maps


def kernel(x, w_q, w_k, w_v, w_o, _trace=False, _results_out=None):
    x = np.asarray(x, dtype=np.float32)
    w_q = np.asarray(w_q, dtype=np.float32)
    w_k = np.asarray(w_k, dtype=np.float32)
    w_v = np.asarray(w_v, dtype=np.float32)
    w_o = np.asarray(w_o, dtype=np.float32)
    nc = _build()
    in_maps = _host_inputs(x, w_q, w_k, w_v, w_o)
    res = run_bass_kernel_spmd(
        nc, in_maps, core_ids=list(range(N_CORES)), trace=_trace)
    if _results_out is not None:
        _results_out.append(res)
    out = np.empty((B, S, D), np.float32)
    for b in range(B):
        acc = res.results[4 * b]["resT"].astype(np.float32)
        for g in range(1, 4):
            acc = acc + res.results[4 * b + g]["resT"].astype(np.float32)
        out[b] = acc.T
    return out


# revision 29
# speedup vs baseline: 1.0227x; 1.0227x over previous
"""Llama MHA (B=2, S=2048, D=2048, H=16, causal, RoPE) on 8 trn2 cores.

Sharding: data-parallel over batch (2 groups of 4 cores) x tensor-parallel
over heads (4 heads per core). Each core computes, for its (batch, 4 heads):
  qT/kT = w^T x^T  (features on partitions, seq on free dim)
  RoPE on qT/kT (weights column-permuted on host so even/odd feature pairs
  land de-interleaved; dot products are permutation-invariant so scores
  match the reference exactly). RoPE runs as one Scalar-engine PSUM drain
  to fp16 followed by all-fp16 SBUF DVE ops (4x DVE mode).
  scoresT[k,q] blocks -> exp(fp16). Causal handling: diagonal-superblock
  matmuls are narrowed to the valid query range (saves ~10% of attention
  PE cycles) and a single shared [128,128] triangular mask covers the
  128-wide diagonal window of each key block.
  Softmax denominator: pt blocks are accumulated per (head, qchunk) with
  cheap fp16 DVE adds, then ONE [128,128]-ones matmul per (head, qchunk)
  does the partition reduction with a free PSUM row-broadcast (replaces
  the per-block ones-matmuls: ~82K -> ~8K PE cycles).
  Scheduling: the attention block loop is exp-throughput-bound on the
  Scalar engine, so chunk qc's out-projection is interleaved at block
  granularity into chunk qc+1's attention, each block's PV matmul is
  deferred one block (depth-2 software pipeline), and chunk 0's attention
  is interleaved into the projection of chunks 1-2. Scratch matmuls keep
  the PE's HAM activity monitor warm through the DMA-bound ramp. Partial
  results ship as fp16 and the host sums the 4 partials per batch in fp32.

All matmuls in fp16 (fp32 PSUM accumulation); softmax/normalization fp32.
"""

import numpy as np

import concourse.bass as bass
import concourse.mybir as mybir
import concourse.tile as tile
from concourse import bacc
from concourse.bass_utils import run_bass_kernel_spmd

B, S, D, H = 2, 2048, 2048, 16
DH = D // H            # 128 head dim
HPC = 4                # heads per core
N_CORES = 8
FH = HPC * DH          # 512 features per core
P = 128
KT = D // P            # 16 k-tiles over D
SC = S // 512          # 4 seq chunks of 512
ST = S // P            # 16 seq blocks of 128
THETA = 10000.0
SCALE = 1.0 / np.sqrt(DH)

DT = mybir.dt.float16
NPDT = np.float16

_prog_cache = {}


def _build():
    if "nc" in _prog_cache:
        return _prog_cache["nc"]
    nc = bacc.Bacc(None, target_bir_lowering=False, debug=False)

    xT = nc.dram_tensor("xT", [D, S], DT, kind="ExternalInput")
    wq = nc.dram_tensor("wq", [D, FH], DT, kind="ExternalInput")
    wk = nc.dram_tensor("wk", [D, FH], DT, kind="ExternalInput")
    wv = nc.dram_tensor("wv", [D, FH], DT, kind="ExternalInput")
    wo = nc.dram_tensor("wo", [FH, D], DT, kind="ExternalInput")
    cc = nc.dram_tensor("cc", [P, S], DT, kind="ExternalInput")
    ss = nc.dram_tensor("ss", [P, S], DT, kind="ExternalInput")
    masks = nc.dram_tensor("masks", [P, P], DT, kind="ExternalInput")
    resT = nc.dram_tensor("resT", [D, S], DT, kind="ExternalOutput")

    f32 = mybir.dt.float32
    Exp = mybir.ActivationFunctionType.Exp
    Copy = mybir.ActivationFunctionType.Copy

    with tile.TileContext(nc) as tc:
        with (
            tc.tile_pool(name="persist", bufs=1) as pp,
            tc.tile_pool(name="psA", bufs=5, space="PSUM") as psA,
            tc.tile_pool(name="psO", bufs=2, space="PSUM") as psO,
        ):
            qT = pp.tile([P, HPC, S], DT)     # per head: rows=feat, free=seq
            kT = pp.tile([P, HPC, S], DT)
            vn = pp.tile([P, ST, FH], DT)     # v natural: [seq-block, feat]
            attnT = pp.tile([P, HPC, S], DT)  # normalized attention output^T
            cc_t = pp.tile([P, S], DT)
            ss_t = pp.tile([P, S], DT)
            mask_t = pp.tile([P, P], DT)      # tri mask: 1 if k <= q (local)
            ones_mat = pp.tile([P, P], DT)    # denominator reduce stationary
            wo_t = pp.tile([P, HPC, D], DT)

            nc.vector.memset(ones_mat, 1.0)

            # ---------------- Phase 1: projections + RoPE -----------------
            with (
                tc.tile_pool(name="wpool", bufs=1) as wp,
                tc.tile_pool(name="xpool", bufs=2) as xp,
                tc.tile_pool(name="ropetmp", bufs=6) as rp,
                tc.tile_pool(name="warm", bufs=1) as wmp,
            ):
                # HAM pre-warm: full-width matmuls on scratch data keep the
                # PE array genuinely busy through the ~3.4us activity window
                # while the input DMAs stage, so the first real matmuls run
                # at full clock. (Narrow matmuls don't register enough
                # activity — the array idles between dispatches.)
                wsb = wmp.tile([P, 256], DT)
                wps = psA.tile([P, 256], f32, tag="ps", name="warm")
                nc.vector.memset(wsb, 0.0)
                for _ in range(16):
                    nc.tensor.matmul(wps, wsb[:, 0:P], wsb, start=True, stop=True)

                wq_t = wp.tile([P, KT, FH], DT)
                wk_t = wp.tile([P, KT, FH], DT)
                wv_t = wp.tile([P, KT, FH], DT)
                # DMA issue order is the Sync-queue order: stage the first x
                # chunk and wq head-by-head so the first q-chain (head 0) can
                # start after ~2.5MB instead of 4MB; defer everything not
                # needed immediately.
                # Weights stream on the GpSimd DGE queue, activations on the
                # SP queue — two hardware DMA paths in parallel through the
                # DMA-bound ramp.
                xc0 = xp.tile([P, KT, 512], DT, tag="xc", name="xc0")
                wqr = wq.rearrange("(kt p) f -> p kt f", p=P)
                for gs in (slice(0, 2), slice(2, 4), slice(4, 8),
                           slice(8, 12), slice(12, 16)):
                    nc.gpsimd.dma_start(out=wq_t[:, gs, 0:DH], in_=wqr[:, gs, 0:DH])
                    nc.sync.dma_start(
                        out=xc0[:, gs, :],
                        in_=xT.rearrange("(kt p) s -> p kt s", p=P)[:, gs, 0:512])
                for h in range(1, HPC):
                    fsl = slice(h * DH, (h + 1) * DH)
                    nc.gpsimd.dma_start(out=wq_t[:, :, fsl], in_=wqr[:, :, fsl])
                nc.sync.dma_start(out=cc_t[:, 0:512], in_=cc[:, 0:512])
                nc.sync.dma_start(out=ss_t[:, 0:512], in_=ss[:, 0:512])
                for g in range(4):
                    gs = slice(g * 4, (g + 1) * 4)
                    nc.gpsimd.dma_start(
                        out=wk_t[:, gs, :],
                        in_=wk.rearrange("(kt p) f -> p kt f", p=P)[:, gs, :])
                nc.gpsimd.dma_start(
                    out=wv_t, in_=wv.rearrange("(kt p) f -> p kt f", p=P))
                nc.sync.dma_start(out=mask_t, in_=masks[:, :])
                nc.sync.dma_start(out=cc_t[:, 512:], in_=cc[:, 512:])
                nc.sync.dma_start(out=ss_t[:, 512:], in_=ss[:, 512:])

                # Attention for query-chunk 0 is interleaved into the
                # projection of chunks 1..2 (it only needs chunk-0 q/k/v,
                # all ready after sc=0). Each block is split into a `pre`
                # (score matmul + exp + mask) and `post` (PV + running
                # denominator) emitted around a full projection chain, so
                # the ~600ns exp hides under ~3.5us of projection matmuls.
                def build_attn0_steps():
                    steps = []
                    for h in range(HPC):
                        fsl = slice(h * DH, (h + 1) * DH)
                        st = {}
                        for kb in range(4):
                            off = 128 * kb if kb > 0 else 0
                            wsl = slice(off, 512)

                            def pre(h=h, kb=kb, off=off, wsl=wsl, st=st):
                                if kb == 0:
                                    st["po"] = psO.tile(
                                        [P, 512], f32, tag="po", name=f"a0po{h}")
                                    st["pda"] = rp.tile(
                                        [P, 512], DT, tag="pda0", bufs=2,
                                        name=f"a0pda{h}")
                                ps = psA.tile([P, 512], f32, tag="ps",
                                              name=f"a0ps{h}{kb}")
                                nc.tensor.matmul(
                                    ps[:, wsl], kT[:, h, kb * P:(kb + 1) * P],
                                    qT[:, h, off:512], start=True, stop=True)
                                pt = rp.tile([P, 512], DT, tag="pt0", bufs=4,
                                             name=f"a0pt{h}{kb}")
                                nc.scalar.activation(
                                    pt[:, wsl], ps[:, wsl], Exp,
                                    scale=float(SCALE))
                                nc.vector.tensor_mul(
                                    pt[:, off:off + P], pt[:, off:off + P],
                                    mask_t)
                                st["pt"] = pt

                            def post(h=h, kb=kb, off=off, wsl=wsl, st=st,
                                     fsl=fsl):
                                pt, po, pda = st["pt"], st["po"], st["pda"]
                                nc.tensor.matmul(
                                    po[:, wsl], vn[:, kb, fsl], pt[:, wsl],
                                    start=(kb == 0), stop=(kb == 3),
                                    skip_group_check=True)
                                if kb == 0:
                                    nc.vector.tensor_copy(pda, pt)
                                else:
                                    nc.vector.tensor_add(
                                        pda[:, wsl], pda[:, wsl], pt[:, wsl])

                            steps.append((pre, post))

                        def fin(h=h, st=st):
                            pd = psO.tile([P, 512], f32, tag="pd", bufs=1,
                                          name=f"a0pd{h}")
                            nc.tensor.matmul(pd, ones_mat, st["pda"],
                                             start=True, stop=True)
                            bc = rp.tile([P, 512], f32, tag="bc0", bufs=2,
                                         name=f"a0bc{h}")
                            nc.vector.reciprocal_approx_fast(out=bc, in_=pd)
                            nc.vector.tensor_mul(
                                attnT[:, h, 0:512], st["po"], bc)

                        steps.append((fin, None))
                    return steps

                attn0_steps = build_attn0_steps()
                attn0_idx = [0]

                def attn0_pre():
                    if attn0_idx[0] < len(attn0_steps):
                        attn0_steps[attn0_idx[0]][0]()

                def attn0_post():
                    if attn0_idx[0] < len(attn0_steps):
                        post = attn0_steps[attn0_idx[0]][1]
                        if post is not None:
                            post()
                        attn0_idx[0] += 1

                for sc in range(SC):
                    if sc == 0:
                        xc = xc0
                    else:
                        xc = xp.tile([P, KT, 512], DT, tag="xc", name=f"xc{sc}")
                        for g in range(4):
                            gs = slice(g * 4, (g + 1) * 4)
                            nc.sync.dma_start(
                                out=xc[:, gs, :],
                                in_=xT.rearrange("(kt p) s -> p kt s", p=P)[
                                    :, gs, sc * 512:(sc + 1) * 512],
                            )
                    if sc == 1:
                        nc.gpsimd.dma_start(
                            out=wo_t, in_=wo.rearrange("(ft p) d -> p ft d", p=P))
                    csl = slice(sc * 512, (sc + 1) * 512)
                    # q/k projections. RoPE fused into the PSUM drain:
                    # Scalar engine copies PSUM->fp16 SBUF, then three
                    # all-fp16 DVE ops (eligible for the fast DVE mode).
                    # ss_t rows 0:64 = +sin, rows 64:128 = -sin; the swap is
                    # done by writing each product into the opposite half.
                    for ci, (wt, dst, h) in enumerate(
                            [(w, d, hh) for w, d in ((wq_t, qT), (wk_t, kT))
                             for hh in range(HPC)]):
                        fsl = slice(h * DH, (h + 1) * DH)
                        if sc >= 1:
                            attn0_pre()
                        pq = psA.tile([P, 512], f32, tag="ps", name=f"pq{sc}{h}{ci}")
                        for k in range(KT):
                            nc.tensor.matmul(
                                pq, wt[:, k, fsl], xc[:, k, :],
                                start=(k == 0), stop=(k == KT - 1),
                            )
                            if sc == 0 and ci < 4 and k % 4 == 3:
                                # scratch matmul: keeps HAM warm through the
                                # DMA-bound stretch of the first chains
                                nc.tensor.matmul(wps, wsb[:, 0:P], wsb,
                                                 start=True, stop=True)
                        if sc >= 1:
                            attn0_post()
                        pqs = rp.tile([P, 512], DT, tag="pqs")
                        ta = rp.tile([P, 512], DT, tag="ta")
                        tb = rp.tile([P, 512], DT, tag="tb")
                        nc.scalar.activation(pqs, pq, Copy)
                        nc.vector.tensor_mul(ta, pqs, cc_t[:, csl])
                        nc.vector.tensor_mul(
                            tb[0:64, :], pqs[64:128, :], ss_t[64:128, csl])
                        nc.vector.tensor_mul(
                            tb[64:128, :], pqs[0:64, :], ss_t[0:64, csl])
                        nc.vector.tensor_add(dst[:, h, csl], ta, tb)
                    # v projection straight into natural layout
                    for st4 in range(4):
                        sb = sc * 4 + st4
                        if sc >= 1:
                            attn0_pre()
                        pv = psA.tile([P, FH], f32, tag="ps", name=f"pv{sc}{st4}")
                        for k in range(KT):
                            nc.tensor.matmul(
                                pv, xc[:, k, st4 * P:(st4 + 1) * P], wv_t[:, k, :],
                                start=(k == 0), stop=(k == KT - 1),
                            )
                        if sc >= 1:
                            attn0_post()
                        nc.vector.tensor_copy(vn[:, sb, :], pv)
                assert attn0_idx[0] >= len(attn0_steps), "attn0 steps left over"

            # ---------- Phase 2+3: attention + interleaved out-proj -------
            # The attention inner loop is exp-throughput-bound on the Scalar
            # engine (~600ns/block vs ~430ns of PE work), so the PE would
            # stall on every PV matmul waiting for its exp. The previous
            # chunk's out-projection is dependency-free by then: its db
            # blocks are injected INTO the block loop (one per few blocks)
            # to fill those stalls. The final chunk's out-projection runs
            # standalone at the end (nothing left to overlap with).
            with (
                tc.tile_pool(name="ppool", bufs=8) as ptp,
                tc.tile_pool(name="rpool", bufs=2) as rop,
            ):
                resTr = resT.rearrange("(db p) s -> p db s", p=P)

                def outproj_units(qc):
                    """One unit per db: 4 matmuls + drain (+ paired DMA)."""
                    qsl = slice(qc * 512, (qc + 1) * 512)
                    rt = rop.tile([P, KT, 512], DT, tag="rt", name=f"rt{qc}")

                    def unit(db):
                        pr = psA.tile([P, 512], f32, tag="ps", name=f"pr{qc}{db}")
                        for ft in range(HPC):
                            nc.tensor.matmul(
                                pr, wo_t[:, ft, db * P:(db + 1) * P],
                                attnT[:, ft, qsl],
                                start=(ft == 0), stop=(ft == HPC - 1),
                            )
                        if db % 4 == 3:
                            nc.scalar.activation(rt[:, db, :], pr, Copy)
                        else:
                            nc.vector.tensor_copy(rt[:, db, :], pr)
                        if db % 2 == 1:
                            nc.sync.dma_start(
                                out=resTr[:, db - 1:db + 1, qsl],
                                in_=rt[:, db - 1:db + 1, :])

                    return [lambda db=db: unit(db) for db in range(KT)]

                # Depth-2 software pipeline: block k's PV matmul is emitted
                # during block k+1 (after its score matmul and any injected
                # out-proj unit), so the PE never sits in-order behind the
                # ~600ns exp. Each head's denominator finalize (ones-matmul,
                # reciprocal, normalize) is likewise deferred into the next
                # head. `pend` carries across heads and chunks.
                pend = {"pv": None, "fin": None}

                def flush_pv():
                    if pend["pv"] is not None:
                        pend["pv"]()
                        pend["pv"] = None

                def flush_fin():
                    if pend["fin"] is not None:
                        pend["fin"]()
                        pend["fin"] = None

                def attention_chunk(qc, inject):
                    qsl = slice(qc * 512, (qc + 1) * 512)
                    nkb = 4 * qc + 4
                    nblocks = HPC * nkb
                    # spread the injected units evenly across the block loop
                    # start injecting a few blocks in: the first unit's ft=3
                    # matmul needs the previous chunk's last-head normalize
                    # (~1.5us of DVE latency past its ones-matmul) to finish
                    inj_at = {}
                    for i, u in enumerate(inject):
                        inj_at.setdefault(
                            min(nblocks - 1, (i * nblocks) // len(inject) + 3),
                            []).append(u)
                    blk = 0
                    for h in range(HPC):
                        fsl = slice(h * DH, (h + 1) * DH)
                        po = psO.tile([P, 512], f32, tag="po", name=f"po{h}{qc}")
                        pda = ptp.tile([P, 512], DT, tag="pda", bufs=3,
                                       name=f"pda{h}{qc}")
                        for kb in range(nkb):
                            jl = kb - 4 * qc       # >=0 only on the diagonal
                            off = 128 * jl if jl > 0 else 0
                            wsl = slice(off, 512)
                            ps = psA.tile([P, 512], f32, tag="ps",
                                          name=f"ps{h}{qc}{kb}")
                            nc.tensor.matmul(
                                ps[:, wsl], kT[:, h, kb * P:(kb + 1) * P],
                                qT[:, h, qc * 512 + off:(qc + 1) * 512],
                                start=True, stop=True,
                            )
                            pt = ptp.tile([P, 512], DT, tag="pt")
                            nc.scalar.activation(
                                pt[:, wsl], ps[:, wsl], Exp, scale=float(SCALE))
                            if jl >= 0:
                                # triangular mask on the 128-wide diag window
                                nc.vector.tensor_mul(
                                    pt[:, off:off + P], pt[:, off:off + P],
                                    mask_t)
                            flush_pv()
                            if kb == 1:
                                # previous head's/chunk's finalize runs (and
                                # writes attnT) before any injected out-proj
                                # unit can read that attnT chunk
                                flush_fin()
                            for u in inj_at.get(blk, ()):
                                u()

                            def pv_unit(po=po, pda=pda, pt=pt, kb=kb, wsl=wsl,
                                        fsl=fsl, nkb=nkb):
                                nc.tensor.matmul(
                                    po[:, wsl], vn[:, kb, fsl], pt[:, wsl],
                                    start=(kb == 0), stop=(kb == nkb - 1),
                                    skip_group_check=True,
                                )
                                if kb == 0:
                                    nc.vector.tensor_copy(pda, pt)
                                else:
                                    nc.vector.tensor_add(
                                        pda[:, wsl], pda[:, wsl], pt[:, wsl])

                            pend["pv"] = pv_unit
                            blk += 1

                        def fin_unit(h=h, qc=qc, po=po, pda=pda, qsl=qsl):
                            pd = psO.tile([P, 512], f32, tag="pd", bufs=1,
                                          name=f"pd{h}{qc}")
                            nc.tensor.matmul(pd, ones_mat, pda,
                                             start=True, stop=True)
                            bc = ptp.tile([P, 512], f32, tag="bc", bufs=4)
                            nc.vector.reciprocal_approx_fast(out=bc, in_=pd)
                            nc.vector.tensor_mul(attnT[:, h, qsl], po, bc)

                        flush_fin()
                        pend["fin"] = fin_unit

                # chunk 0's attention already ran inside phase 1
                for qc in range(1, SC):
                    attention_chunk(qc, outproj_units(qc - 1))
                flush_pv()
                flush_fin()
                for u in outproj_units(SC - 1):
                    u()

    nc.finalize()
    _prog_cache["nc"] = nc
    return nc


def _host_inputs(x, w_q, w_k, w_v, w_o):
    """Build the 8 per-core input maps."""
    # RoPE de-interleave permutation per head: evens then odds
    i = np.arange(DH)
    perm_head = np.concatenate([i[0::2], i[1::2]])  # within-head column order

    t = np.arange(S, dtype=np.float64)
    inv_freq = 1.0 / (THETA ** (np.arange(0, DH, 2, dtype=np.float64) / DH))
    ang = np.outer(t, inv_freq)          # [S, 64]
    cosT = np.cos(ang).T.astype(np.float32)   # [64, S]
    sinT = np.sin(ang).T.astype(np.float32)
    cc = np.vstack([cosT, cosT]).astype(NPDT)   # [128, S]
    ss = np.vstack([sinT, -sinT]).astype(NPDT)  # +sin feeds bottom half

    # shared diagonal mask: mask[k, q] = 1 if k <= q (128-wide local window)
    kk = np.arange(P)[:, None]
    qq = np.arange(P)[None, :]
    masks = (kk <= qq).astype(NPDT)      # [128, 128]

    in_maps = []
    for core in range(N_CORES):
        b = core // 4
        h0 = (core % 4) * HPC
        cols = np.concatenate(
            [h * DH + perm_head for h in range(h0, h0 + HPC)])   # rope-permuted
        vcols = np.arange(h0 * DH, (h0 + HPC) * DH)              # natural
        in_maps.append({
            "xT": np.ascontiguousarray(x[b].T).astype(NPDT),
            "wq": np.ascontiguousarray(w_q[:, cols]).astype(NPDT),
            "wk": np.ascontiguousarray(w_k[:, cols]).astype(NPDT),
            "wv": np.ascontiguousarray(w_v[:, vcols]).astype(NPDT),
            "wo": np.ascontiguousarray(w_o[vcols, :]).astype(NPDT),
            "cc": cc,
            "ss": ss,
            "masks": masks,
        })
    return in_maps


def kernel(x, w_q, w_k, w_v, w_o, _trace=False, _results_out=None):
    x = np.asarray(x, dtype=np.float32)
    w_q = np.asarray(w_q, dtype=np.float32)
    w_k = np.asarray(w_k, dtype=np.float32)
    w_v = np.asarray(w_v, dtype=np.float32)
    w_o = np.asarray(w_o, dtype=np.float32)
    nc = _build()
    in_maps = _host_inputs(x, w_q, w_k, w_v, w_o)
    res = run_bass_kernel_spmd(
        nc, in_maps, core_ids=list(range(N_CORES)), trace=_trace)
    if _results_out is not None:
        _results_out.append(res)
    out = np.empty((B, S, D), np.float32)
    for b in range(B):
        acc = res.results[4 * b]["resT"].astype(np.float32)
        for g in range(1, 4):
            acc = acc + res.results[4 * b + g]["resT"].astype(np.float32)
        out[b] = acc.T
    return out
